# revision 14
# baseline (speedup 1.0000x reference)
"""Grouped-experts SwiGLU MoE kernel for Trainium2 (8 NeuronCores).

Expert-parallel sharding: core e owns expert e's weights and its contiguous
token group (m_sizes gives T//E = 2048 tokens per expert). No collectives —
routing/scatter/gather happens on the host, each core runs an identical
single-core program on its own shard.

Per-core math: out = (silu(x_e @ w1_e) * (x_e @ w3_e)) @ w2_e
  x_e [2048, 2048], w1/w3 [2048, 1024], w2 [1024, 2048].

Device strategy (all matmul operands bf16, f32 PSUM accumulation —
rel(absmax) ≈ 5e-3 vs the f32 reference, well inside the 2e-2 gate):
  phase 1 (up+gate):  stationary = w1/w3 128x128 tiles, moving = xT tiles
      (pre-transposed on host so D is the partition/contraction axis).
      PSUM accumulates over D; SwiGLU evac (ACT silu + DVE mul) writes the
      intermediate zT [H, M] as bf16.
  phase 2 (down):     stationary = zT 128x128 tiles, moving = w2 tiles
      (resident in SBUF). PSUM accumulates over H; DVE copies to SBUF as
      bf16 and DMA stores out [M, D] bf16; the host upcasts to f32.

Scheduling notes (from perfetto traces; the steady-state matmul stream runs
at the 216ns/512-col bf16 issue rate, so everything here is about keeping
the stream gapless):
  - w1 and w3 are interleaved on the host into ONE dram tensor w13r so a
    full h-iteration's weights (1MB) load with a single dma_start. With
    per-(h,cg) issues the ACT queue spent ~600ns per issue and the tile
    scheduler's buffer-reuse waits head-of-line blocked the queue right
    when the phase-1 PSUM evacs (ACT silu) had to run, starving the PE of
    free accumulator banks for ~8us per transition.
  - The weight stream is issued on the ACT HWDGE queue while x / w2 / out
    use the SP queue, so the first weight tiles land in parallel with the
    first xT chunks instead of queueing behind them.
  - PE warm-up: the HAM clock gate runs the PE at 1.2 GHz until ~3.4us of
    sustained busy time and re-throttles after idle windows. 14 dummy
    matmuls on a zeroed tile keep the PE busy from the end of the
    framework preamble (~7.5us) until the first real operands land
    (~13us: the 16 DMA engines take ~3us to wake after the first
    doorbell), so the real stream starts warm and never dips.
  - Half 0's first two h-iterations are fused (8 matmuls per x-chunk): the
    cold DMA subsystem only sustains ~half the steady chunk rate, and
    halving the per-chunk demand removes the startup feed stalls. The
    startup weights are issued in exact consumption order as three waves
    (c=0 sub-tiles for h0/h1 w1/w3, then c=1:4, then the c=4:16 bulk).
  - The last 4 c-chunks of the fused pair run h0-then-h1 (instead of
    interleaved) and h0's evac is emitted immediately after its last
    matmul, so the PSUM banks h2 reuses are free ~3.5us before h1
    finishes and h2 starts without an evac stall.
  - A dummy 1-column silu right after the startup issues hoists the lazy
    ACT_TABLE_LOAD (1.3us) of the silu table out of the first-evac
    critical path.
  - w2's resident load is issued after the fused pair so phase 2 (at
    ~120us) never waits on it and it stays clear of the startup window.
  - The last token block's PSUM evac is interleaved with its final
    matmuls (alternating ACT copy / DVE cast per 512-col bank) and stored
    per 512-col chunk (last store on the ACT HWDGE queue), shortening the
    post-matmul tail to ~2us of evac+store + the fixed ~3us teardown.
Tokens are processed in two halves of 1024 so the 4 PSUM accumulator banks
(2 for u, 2 for g) can ping-pong across h-iterations (bufs=2 -> 8 banks),
keeping the matmul stream free of evac stalls.
"""

import numpy as np
import ml_dtypes

E, T, D, H = 8, 16384, 2048, 1024
M = T // E            # tokens per expert
P = 128
DC = D // P           # 16 contraction chunks (phase 1)
HC = H // P           # 8 contraction chunks (phase 2)
NHALF = 2
MH = M // NHALF       # 1024 tokens per half
NMOV = 512            # moving free dim / PSUM bank width (f32)

_CACHE = {}
LAST_RESULTS = None   # for test harnesses that want the profile


def _build_program():
    import concourse.bacc as bacc
    import concourse.bass as bass
    import concourse.mybir as mybir
    import concourse.tile as tile

    f32 = mybir.dt.float32
    bf16 = mybir.dt.bfloat16
    SILU = mybir.ActivationFunctionType.Silu

    nc = bacc.Bacc("TRN2", target_bir_lowering=False, debug=False)

    xT = nc.dram_tensor("xT", [D, M], bf16, kind="ExternalInput")
    # [h, p, w, c, m] = w{1|3}[c*P+p, h*P+m]; one h-iteration = one slab
    w13r = nc.dram_tensor("w13r", [HC, P, 2, DC, P], bf16, kind="ExternalInput")
    w2r = nc.dram_tensor("w2r", [HC, P, D], bf16, kind="ExternalInput")
    out = nc.dram_tensor("out", [M, D], bf16, kind="ExternalOutput")

    xT_t = xT.rearrange("(c p) m -> p c m", p=P)  # [P, DC, M]

    with tile.TileContext(nc) as tc:
        with (
            tc.tile_pool(name="xp", bufs=1) as xp,
            tc.tile_pool(name="w2p", bufs=1) as w2p,
            tc.tile_pool(name="zp", bufs=1) as zp,
            tc.tile_pool(name="wp", bufs=3) as wp,
            tc.tile_pool(name="op", bufs=2) as op,
            tc.tile_pool(name="sp", bufs=3) as sp,
            tc.tile_pool(name="wub", bufs=1) as wub,
            tc.tile_pool(name="ps", bufs=2, space=bass.MemorySpace.PSUM) as ps,
        ):
            w2t = w2p.tile([P, HC, D], bf16, tag="w2")

            wut = wub.tile([P, NMOV], bf16, tag="wu")
            nc.vector.memset(wut[:], 0)
            pwu = ps.tile([P, NMOV], f32, tag="p0", name="warm")
            NWARM = 14
            for i in range(NWARM):
                nc.tensor.matmul(
                    pwu[:], wut[:, 0:P], wut[:],
                    start=i == 0, stop=i == NWARM - 1,
                )

            for hf in range(NHALF):
                msl = slice(hf * MH, (hf + 1) * MH)
                xt = xp.tile([P, DC, MH], bf16, tag="xt")
                for c in range(DC):
                    # half-0 loads race the first weight tiles (on the other
                    # DGE queue); half-1 loads overlap half-0 phase 2
                    if hf == 0 and c == 0:
                        # first chunk in two pieces: the first matmul needs
                        # only 512 columns
                        nc.sync.dma_start(xt[:, 0, 0:NMOV], xT_t[:, 0, 0:NMOV])
                        nc.sync.dma_start(xt[:, 0, NMOV:MH], xT_t[:, 0, NMOV:MH])
                    else:
                        nc.sync.dma_start(xt[:, c, :], xT_t[:, c, msl])

                zt = zp.tile([P, HC, MH], bf16, tag="zt")

                # ---- phase 1: u = x@w1, g = x@w3, z = silu(u)*g ----
                def p1_weights_h(h):
                    wt = wp.tile([P, 2, DC, P], bf16, tag="w13", name=f"w13_{hf}_{h}")
                    nc.scalar.dma_start(wt[:], w13r[h])
                    return wt

                def p1_matmuls(pu, pg, wt, c):
                    first, last = c == 0, c == DC - 1
                    for mi in range(MH // NMOV):
                        nc.tensor.matmul(
                            pu[mi][:], wt[:, 0, c, :],
                            xt[:, c, mi * NMOV:(mi + 1) * NMOV],
                            start=first, stop=last,
                        )
                    for mi in range(MH // NMOV):
                        nc.tensor.matmul(
                            pg[mi][:], wt[:, 1, c, :],
                            xt[:, c, mi * NMOV:(mi + 1) * NMOV],
                            start=first, stop=last,
                        )

                def p1_evac(h, pu, pg):
                    for mi in range(MH // NMOV):
                        st = sp.tile([P, NMOV], bf16, tag="st")
                        nc.scalar.activation(st[:], pu[mi][:], SILU)
                        nc.vector.tensor_mul(
                            zt[:, h, mi * NMOV:(mi + 1) * NMOV],
                            st[:], pg[mi][:],
                        )

                def p1_banks():
                    pu = [ps.tile([P, NMOV], f32, tag=f"p{i}", name=f"pu{i}") for i in range(2)]
                    pg = [ps.tile([P, NMOV], f32, tag=f"p{i + 2}", name=f"pg{i}") for i in range(2)]
                    return pu, pg

                if hf == 0:
                    acc = [p1_banks(), p1_banks()]
                    # startup weights in exact consumption order: the four
                    # c=0 sub-tiles (h0 w1/w3, then h1 w1/w3, 32KB each)
                    # land first so matmuls 1-8 never block behind a bulk
                    # transfer; c=1:4 then c=4:16 follow.
                    wts0 = [
                        wp.tile([P, 2, DC, P], bf16, tag="w13", name=f"w13s{h}")
                        for h in range(2)
                    ]
                    # 128KB max per issue: a slab's semaphore only fires when
                    # ALL of it lands, so big slabs make their first c-chunk
                    # as late as their last. The c>=8 waves are emitted after
                    # the first matmuls (lower scheduler priority) so their
                    # transfers don't crowd the x chunks c1-c4 off the DMA
                    # engines during the 13-20us window.
                    for csl in (slice(0, 1), slice(1, 4)):
                        for h in range(2):
                            for w in range(2):
                                nc.scalar.dma_start(
                                    wts0[h][:, w, csl, :], w13r[h, :, w, csl, :]
                                )
                    # hoist the lazy silu ACT_TABLE_LOAD (1.3us) out of the
                    # first-evac critical path; runs off-critical at ~15us
                    dst = sp.tile([P, NMOV], bf16, tag="st", name="dummy_silu")
                    nc.scalar.activation(dst[:, 0:1], wut[:, 0:1], SILU)

                    for c in range(0, 12):
                        for h in range(2):
                            p1_matmuls(*acc[h], wts0[h], c)
                        if c == 0:
                            for h2_ in range(2):
                                for w in range(2):
                                    nc.scalar.dma_start(
                                        wts0[h2_][:, w, 4:8, :],
                                        w13r[h2_, :, w, 4:8, :],
                                    )
                        elif c == 2:
                            for csl in (slice(8, 12), slice(12, DC)):
                                for h2_ in range(2):
                                    for w in range(2):
                                        nc.scalar.dma_start(
                                            wts0[h2_][:, w, csl, :],
                                            w13r[h2_, :, w, csl, :],
                                        )
                    # last 4 chunks: finish ALL of h0 first and evac it
                    # immediately, so the PSUM banks h2 reuses are free
                    # ~3.5us before h1's last matmul.
                    for h in range(2):
                        for c in range(12, DC):
                            p1_matmuls(*acc[h], wts0[h], c)
                        if h == 0:
                            p1_evac(0, *acc[0])
                    p1_evac(1, *acc[1])
                    # w2 resident for the whole kernel; issued once the
                    # critical startup window is past (needed only at ~120us)
                    for hh in range(HC):
                        nc.sync.dma_start(w2t[:, hh, :], w2r[hh])
                    h_rest = range(2, HC)
                else:
                    h_rest = range(HC)

                for h in h_rest:
                    wt = p1_weights_h(h)
                    pu, pg = p1_banks()
                    for c in range(DC):
                        p1_matmuls(pu, pg, wt, c)
                    p1_evac(h, pu, pg)

                # ---- phase 2: out = z @ w2 ----
                for mi in range(MH // P):
                    po = [ps.tile([P, NMOV], f32, tag=f"p{dd}", name=f"po{dd}") for dd in range(4)]
                    osb = op.tile([P, D], bf16, tag="o")
                    r0 = hf * MH + mi * P
                    last_blk = hf == NHALF - 1 and mi == MH // P - 1
                    if not last_blk:
                        for h in range(HC):
                            zst = zt[:, h, mi * P:(mi + 1) * P]
                            for dd in range(D // NMOV):
                                nc.tensor.matmul(
                                    po[dd][:], zst,
                                    w2t[:, h, dd * NMOV:(dd + 1) * NMOV],
                                    start=h == 0, stop=h == HC - 1,
                                )
                        for dd in range(D // NMOV):
                            nc.vector.tensor_copy(
                                osb[:, dd * NMOV:(dd + 1) * NMOV], po[dd][:]
                            )
                        nc.sync.dma_start(out[r0:r0 + P, :], osb[:])
                    else:
                        # last token block: banks 0-2 finish their h-loop
                        # 1.7us before bank 3's trailing column, so their
                        # copies/stores pipeline UNDER bank 3's matmuls; the
                        # final bank's copy is split across ACT and DVE (both
                        # idle by then) and its 128KB store rides the ACT
                        # HWDGE queue. Tail after the last matmul: ~0.35us
                        # copy + 0.6us issue + transfer, vs ~1.9us before.
                        for h in range(HC):
                            zst = zt[:, h, mi * P:(mi + 1) * P]
                            for dd in range(3):
                                nc.tensor.matmul(
                                    po[dd][:], zst,
                                    w2t[:, h, dd * NMOV:(dd + 1) * NMOV],
                                    start=h == 0, stop=h == HC - 1,
                                )
                        nc.scalar.copy(osb[:, 0:NMOV], po[0][:])
                        nc.vector.tensor_copy(osb[:, NMOV:2 * NMOV], po[1][:])
                        nc.sync.dma_start(
                            out[r0:r0 + P, 0:2 * NMOV], osb[:, 0:2 * NMOV]
                        )
                        nc.scalar.copy(osb[:, 2 * NMOV:3 * NMOV], po[2][:])
                        nc.sync.dma_start(
                            out[r0:r0 + P, 2 * NMOV:3 * NMOV],
                            osb[:, 2 * NMOV:3 * NMOV],
                        )
                        for h in range(HC):
                            nc.tensor.matmul(
                                po[3][:], zt[:, h, mi * P:(mi + 1) * P],
                                w2t[:, h, 3 * NMOV:D],
                                start=h == 0, stop=h == HC - 1,
                            )
                        # separate tiles so Tile doesn't serialize the two
                        # half-copies on a same-tile dependency; both 64KB
                        # stores then issue in parallel on the two HWDGE
                        # queues.
                        HM = NMOV // 2
                        o3a = op.tile([P, HM], bf16, tag="o3a")
                        o3b = op.tile([P, HM], bf16, tag="o3b")
                        nc.scalar.copy(o3a[:], po[3][:, 0:HM])
                        nc.scalar.dma_start(
                            out[r0:r0 + P, 3 * NMOV:3 * NMOV + HM], o3a[:]
                        )
                        nc.vector.tensor_copy(o3b[:], po[3][:, HM:NMOV])
                        nc.sync.dma_start(
                            out[r0:r0 + P, 3 * NMOV + HM:D], o3b[:]
                        )

    nc.compile()
    return nc


def _get_program():
    if "nc" not in _CACHE:
        _CACHE["nc"] = _build_program()
    return _CACHE["nc"]


def _prep_w13(w1e, w3e):
    # [D, H] x2 -> [HC, P, 2, DC, P]; element [h,p,w,c,m] = w{1|3}[c*P+p, h*P+m]
    a = w1e.astype(ml_dtypes.bfloat16).reshape(DC, P, HC, P).transpose(2, 1, 0, 3)
    b = w3e.astype(ml_dtypes.bfloat16).reshape(DC, P, HC, P).transpose(2, 1, 0, 3)
    return np.ascontiguousarray(np.stack([a, b], axis=2))


def _numpy_fallback(x, w1, w2, w3, m_sizes):
    offs = np.concatenate([[0], np.cumsum(np.asarray(m_sizes, dtype=np.int64))])
    out = np.zeros((x.shape[0], w2.shape[2]), dtype=np.float32)
    for e in range(w1.shape[0]):
        xe = x[offs[e]:offs[e + 1]]
        u = xe @ w1[e]
        g = xe @ w3[e]
        z = (u / (1.0 + np.exp(-u))) * g
        out[offs[e]:offs[e + 1]] = z @ w2[e]
    return out


def kernel(x, w1, w2, w3, m_sizes, _trace=False, _trace_kwargs=None):
    global LAST_RESULTS
    x = np.ascontiguousarray(x, dtype=np.float32)
    w1 = np.ascontiguousarray(w1, dtype=np.float32)
    w2 = np.ascontiguousarray(w2, dtype=np.float32)
    w3 = np.ascontiguousarray(w3, dtype=np.float32)
    m = np.asarray(m_sizes, dtype=np.int64)

    expected = (
        x.shape == (T, D)
        and w1.shape == (E, D, H)
        and w2.shape == (E, H, D)
        and w3.shape == (E, D, H)
        and m.shape == (E,)
        and np.all(m == M)
    )
    if not expected:
        return _numpy_fallback(x, w1, w2, w3, m_sizes)

    from concourse.bass_utils import run_bass_kernel_spmd

    nc = _get_program()
    in_maps = []
    for e in range(E):
        in_maps.append({
            "xT": np.ascontiguousarray(
                x[e * M:(e + 1) * M].astype(ml_dtypes.bfloat16).T
            ),
            "w13r": _prep_w13(w1[e], w3[e]),
            "w2r": np.ascontiguousarray(
                w2[e].astype(ml_dtypes.bfloat16).reshape(HC, P, D)
            ),
        })

    res = run_bass_kernel_spmd(
        nc, in_maps, core_ids=list(range(E)),
        trace=_trace, **(_trace_kwargs or {}),
    )
    LAST_RESULTS = res
    return np.concatenate(
        [np.asarray(r["out"]) for r in res.results], axis=0
    ).astype(np.float32)


# revision 20
# speedup vs baseline: 1.0055x; 1.0055x over previous
"""Grouped-experts SwiGLU MoE kernel for Trainium2 (8 NeuronCores).

Expert-parallel sharding: core e owns expert e's weights and its contiguous
token group (m_sizes gives T//E = 2048 tokens per expert). No collectives —
routing/scatter/gather happens on the host, each core runs an identical
single-core program on its own shard.

Per-core math: out = (silu(x_e @ w1_e) * (x_e @ w3_e)) @ w2_e
  x_e [2048, 2048], w1/w3 [2048, 1024], w2 [1024, 2048].

Device strategy (all matmul operands bf16, f32 PSUM accumulation —
rel(absmax) ≈ 5e-3 vs the f32 reference, well inside the 2e-2 gate):
  phase 1 (up+gate):  stationary = w1/w3 128x128 tiles, moving = xT tiles
      (pre-transposed on host so D is the partition/contraction axis).
      PSUM accumulates over D; SwiGLU evac (ACT silu + DVE mul) writes the
      intermediate zT [H, M] as bf16.
  phase 2 (down):     stationary = zT 128x128 tiles, moving = w2 tiles
      (resident in SBUF). PSUM accumulates over H; DVE copies to SBUF as
      bf16 and DMA stores out [M, D] bf16; the host upcasts to f32.

Scheduling notes (from perfetto traces; the steady-state matmul stream runs
at the 216ns/512-col bf16 issue rate, so everything here is about keeping
the stream gapless):
  - w1 and w3 are interleaved on the host into ONE dram tensor w13r so a
    full h-iteration's weights (1MB) load with a single dma_start. With
    per-(h,cg) issues the ACT queue spent ~600ns per issue and the tile
    scheduler's buffer-reuse waits head-of-line blocked the queue right
    when the phase-1 PSUM evacs (ACT silu) had to run, starving the PE of
    free accumulator banks for ~8us per transition.
  - The weight stream is issued on the ACT HWDGE queue while x / w2 / out
    use the SP queue, so the first weight tiles land in parallel with the
    first xT chunks instead of queueing behind them.
  - PE warm-up: the HAM clock gate runs the PE at 1.2 GHz until ~3.4us of
    sustained busy time and re-throttles after idle windows. 14 dummy
    matmuls on a zeroed tile keep the PE busy from the end of the
    framework preamble (~7.5us) until the first real operands land
    (~13us: the 16 DMA engines take ~3us to wake after the first
    doorbell), so the real stream starts warm and never dips.
  - Half 0's first two h-iterations are fused (8 matmuls per x-chunk): the
    cold DMA subsystem only sustains ~half the steady chunk rate, and
    halving the per-chunk demand removes the startup feed stalls. The
    startup weights are issued in exact consumption order as three waves
    (c=0 sub-tiles for h0/h1 w1/w3, then c=1:4, then the c=4:16 bulk).
  - The last 4 c-chunks of the fused pair run h0-then-h1 (instead of
    interleaved) and h0's evac is emitted immediately after its last
    matmul, so the PSUM banks h2 reuses are free ~3.5us before h1
    finishes and h2 starts without an evac stall.
  - A dummy 1-column silu right after the startup issues hoists the lazy
    ACT_TABLE_LOAD (1.3us) of the silu table out of the first-evac
    critical path.
  - w2's resident load is issued after the fused pair so phase 2 (at
    ~120us) never waits on it and it stays clear of the startup window.
  - The last token block's PSUM evac is interleaved with its final
    matmuls (alternating ACT copy / DVE cast per 512-col bank) and stored
    per 512-col chunk (last store on the ACT HWDGE queue), shortening the
    post-matmul tail to ~2us of evac+store + the fixed ~3us teardown.
Tokens are processed in two halves of 1024 so the 4 PSUM accumulator banks
(2 for u, 2 for g) can ping-pong across h-iterations (bufs=2 -> 8 banks),
keeping the matmul stream free of evac stalls.
"""

import numpy as np
import ml_dtypes

E, T, D, H = 8, 16384, 2048, 1024
M = T // E            # tokens per expert
P = 128
DC = D // P           # 16 contraction chunks (phase 1)
HC = H // P           # 8 contraction chunks (phase 2)
NHALF = 2
MH = M // NHALF       # 1024 tokens per half
NMOV = 512            # moving free dim / PSUM bank width (f32)

_CACHE = {}
LAST_RESULTS = None   # for test harnesses that want the profile


def _build_program():
    import concourse.bacc as bacc
    import concourse.bass as bass
    import concourse.mybir as mybir
    import concourse.tile as tile

    f32 = mybir.dt.float32
    bf16 = mybir.dt.bfloat16
    SILU = mybir.ActivationFunctionType.Silu

    nc = bacc.Bacc("TRN2", target_bir_lowering=False, debug=False)

    xT = nc.dram_tensor("xT", [D, M], bf16, kind="ExternalInput")
    # [h, p, w, c, m] = w{1|3}[c*P+p, h*P+m]; one h-iteration = one slab
    w13r = nc.dram_tensor("w13r", [HC, P, 2, DC, P], bf16, kind="ExternalInput")
    w2r = nc.dram_tensor("w2r", [HC, P, D], bf16, kind="ExternalInput")
    out = nc.dram_tensor("out", [M, D], bf16, kind="ExternalOutput")

    xT_t = xT.rearrange("(c p) m -> p c m", p=P)  # [P, DC, M]

    with tile.TileContext(nc) as tc:
        with (
            tc.tile_pool(name="xp", bufs=1) as xp,
            tc.tile_pool(name="w2p", bufs=1) as w2p,
            tc.tile_pool(name="zp", bufs=1) as zp,
            tc.tile_pool(name="wp", bufs=3) as wp,
            tc.tile_pool(name="op", bufs=2) as op,
            tc.tile_pool(name="sp", bufs=3) as sp,
            tc.tile_pool(name="wub", bufs=1) as wub,
            tc.tile_pool(name="ps", bufs=2, space=bass.MemorySpace.PSUM) as ps,
        ):
            w2t = w2p.tile([P, HC, D], bf16, tag="w2")

            wut = wub.tile([P, NMOV], bf16, tag="wu")
            nc.vector.memset(wut[:], 0)
            pwu = ps.tile([P, NMOV], f32, tag="p0", name="warm")
            NWARM = 12
            for i in range(NWARM):
                nc.tensor.matmul(
                    pwu[:], wut[:, 0:P], wut[:],
                    start=i == 0, stop=i == NWARM - 1,
                )

            for hf in range(NHALF):
                msl = slice(hf * MH, (hf + 1) * MH)
                xt = xp.tile([P, DC, MH], bf16, tag="xt")
                for c in range(DC):
                    # half-0 loads race the first weight tiles (on the other
                    # DGE queue); half-1 loads overlap half-0 phase 2
                    if hf == 0 and c <= 3:
                        # early chunks in two pieces: their semaphores are
                        # all-or-nothing, and during the cold-DMA window the
                        # first 512 columns are enough for the first matmuls
                        # of the chunk (interleaved mi order below)
                        nc.sync.dma_start(xt[:, c, 0:NMOV], xT_t[:, c, 0:NMOV])
                        nc.sync.dma_start(xt[:, c, NMOV:MH], xT_t[:, c, NMOV:MH])
                    else:
                        nc.sync.dma_start(xt[:, c, :], xT_t[:, c, msl])

                zt = zp.tile([P, HC, MH], bf16, tag="zt")

                # ---- phase 1: u = x@w1, g = x@w3, z = silu(u)*g ----
                def p1_weights_h(h):
                    wt = wp.tile([P, 2, DC, P], bf16, tag="w13", name=f"w13_{hf}_{h}")
                    nc.scalar.dma_start(wt[:], w13r[h])
                    return wt

                def p1_matmuls(pu, pg, wt, c, interleave=False):
                    # interleave: mi-major order so the first 512-col piece
                    # of a split x chunk feeds 2 matmuls before the second
                    # piece is needed (startup only)
                    first, last = c == 0, c == DC - 1
                    if interleave:
                        for mi in range(MH // NMOV):
                            xs = xt[:, c, mi * NMOV:(mi + 1) * NMOV]
                            nc.tensor.matmul(
                                pu[mi][:], wt[:, 0, c, :], xs,
                                start=first, stop=last,
                            )
                            nc.tensor.matmul(
                                pg[mi][:], wt[:, 1, c, :], xs,
                                start=first, stop=last,
                            )
                        return
                    for mi in range(MH // NMOV):
                        nc.tensor.matmul(
                            pu[mi][:], wt[:, 0, c, :],
                            xt[:, c, mi * NMOV:(mi + 1) * NMOV],
                            start=first, stop=last,
                        )
                    for mi in range(MH // NMOV):
                        nc.tensor.matmul(
                            pg[mi][:], wt[:, 1, c, :],
                            xt[:, c, mi * NMOV:(mi + 1) * NMOV],
                            start=first, stop=last,
                        )

                def p1_evac(h, pu, pg):
                    for mi in range(MH // NMOV):
                        st = sp.tile([P, NMOV], bf16, tag="st")
                        nc.scalar.activation(st[:], pu[mi][:], SILU)
                        nc.vector.tensor_mul(
                            zt[:, h, mi * NMOV:(mi + 1) * NMOV],
                            st[:], pg[mi][:],
                        )

                def p1_banks():
                    pu = [ps.tile([P, NMOV], f32, tag=f"p{i}", name=f"pu{i}") for i in range(2)]
                    pg = [ps.tile([P, NMOV], f32, tag=f"p{i + 2}", name=f"pg{i}") for i in range(2)]
                    return pu, pg

                if hf == 0:
                    acc = [p1_banks(), p1_banks()]
                    # startup weights in exact consumption order: the four
                    # c=0 sub-tiles (h0 w1/w3, then h1 w1/w3, 32KB each)
                    # land first so matmuls 1-8 never block behind a bulk
                    # transfer; c=1:4 then c=4:16 follow.
                    wts0 = [
                        wp.tile([P, 2, DC, P], bf16, tag="w13", name=f"w13s{h}")
                        for h in range(2)
                    ]
                    # 128KB max per issue: a slab's semaphore only fires when
                    # ALL of it lands, so big slabs make their first c-chunk
                    # as late as their last. The c>=8 waves are emitted after
                    # the first matmuls (lower scheduler priority) so their
                    # transfers don't crowd the x chunks c1-c4 off the DMA
                    # engines during the 13-20us window.
                    for csl in (slice(0, 1), slice(1, 4)):
                        for h in range(2):
                            for w in range(2):
                                nc.scalar.dma_start(
                                    wts0[h][:, w, csl, :], w13r[h, :, w, csl, :]
                                )
                    # hoist the lazy silu ACT_TABLE_LOAD (1.3us) out of the
                    # first-evac critical path; runs off-critical at ~15us
                    dst = sp.tile([P, NMOV], bf16, tag="st", name="dummy_silu")
                    nc.scalar.activation(dst[:, 0:1], wut[:, 0:1], SILU)

                    for c in range(0, 12):
                        for h in range(2):
                            p1_matmuls(*acc[h], wts0[h], c, interleave=c <= 3)
                        if c == 0:
                            for h2_ in range(2):
                                for w in range(2):
                                    nc.scalar.dma_start(
                                        wts0[h2_][:, w, 4:8, :],
                                        w13r[h2_, :, w, 4:8, :],
                                    )
                        elif c == 2:
                            for csl in (slice(8, 12), slice(12, DC)):
                                for h2_ in range(2):
                                    for w in range(2):
                                        nc.scalar.dma_start(
                                            wts0[h2_][:, w, csl, :],
                                            w13r[h2_, :, w, csl, :],
                                        )
                    # last 4 chunks: finish ALL of h0 first and evac it
                    # immediately, so the PSUM banks h2 reuses are free
                    # ~3.5us before h1's last matmul.
                    for h in range(2):
                        for c in range(12, DC):
                            p1_matmuls(*acc[h], wts0[h], c)
                        if h == 0:
                            p1_evac(0, *acc[0])
                    p1_evac(1, *acc[1])
                    # w2 resident for the whole kernel; issued once the
                    # critical startup window is past (needed only at ~120us)
                    for hh in range(HC):
                        nc.sync.dma_start(w2t[:, hh, :], w2r[hh])
                    h_rest = range(2, HC)
                else:
                    h_rest = range(HC)

                for h in h_rest:
                    wt = p1_weights_h(h)
                    pu, pg = p1_banks()
                    for c in range(DC):
                        p1_matmuls(pu, pg, wt, c)
                    p1_evac(h, pu, pg)

                # ---- phase 2: out = z @ w2 ----
                for mi in range(MH // P):
                    npo = 3 if hf == NHALF - 1 and mi == MH // P - 1 else 4
                    po = [ps.tile([P, NMOV], f32, tag=f"p{dd}", name=f"po{dd}") for dd in range(npo)]
                    osb = op.tile([P, D], bf16, tag="o")
                    r0 = hf * MH + mi * P
                    last_blk = hf == NHALF - 1 and mi == MH // P - 1
                    if not last_blk:
                        for h in range(HC):
                            zst = zt[:, h, mi * P:(mi + 1) * P]
                            for dd in range(D // NMOV):
                                nc.tensor.matmul(
                                    po[dd][:], zst,
                                    w2t[:, h, dd * NMOV:(dd + 1) * NMOV],
                                    start=h == 0, stop=h == HC - 1,
                                )
                        for dd in range(D // NMOV):
                            nc.vector.tensor_copy(
                                osb[:, dd * NMOV:(dd + 1) * NMOV], po[dd][:]
                            )
                        nc.sync.dma_start(out[r0:r0 + P, :], osb[:])
                    else:
                        # last token block: banks 0-2 finish their h-loop
                        # 1.7us before the trailing column, so their
                        # copies/stores pipeline UNDER its matmuls. The last
                        # 512 output columns accumulate as TWO N=256 columns
                        # in two different PSUM banks (same total PE cycles)
                        # so ACT and DVE evacuate them truly in parallel
                        # (same-bank access would serialize) and the two
                        # 64KB stores issue concurrently on both HWDGE
                        # queues. Tail after the last matmul: ~0.35us copy +
                        # 0.6us issue + transfer + fixed teardown.
                        for h in range(HC):
                            zst = zt[:, h, mi * P:(mi + 1) * P]
                            for dd in range(3):
                                nc.tensor.matmul(
                                    po[dd][:], zst,
                                    w2t[:, h, dd * NMOV:(dd + 1) * NMOV],
                                    start=h == 0, stop=h == HC - 1,
                                )
                        nc.scalar.copy(osb[:, 0:NMOV], po[0][:])
                        nc.vector.tensor_copy(osb[:, NMOV:2 * NMOV], po[1][:])
                        nc.sync.dma_start(
                            out[r0:r0 + P, 0:2 * NMOV], osb[:, 0:2 * NMOV]
                        )
                        nc.scalar.copy(osb[:, 2 * NMOV:3 * NMOV], po[2][:])
                        nc.sync.dma_start(
                            out[r0:r0 + P, 2 * NMOV:3 * NMOV],
                            osb[:, 2 * NMOV:3 * NMOV],
                        )
                        HM = NMOV // 2
                        po3a = ps.tile([P, HM], f32, tag="p3", name="po3a")
                        po3b = ps.tile([P, HM], f32, tag="p3", name="po3b")
                        for h in range(HC):
                            zst = zt[:, h, mi * P:(mi + 1) * P]
                            nc.tensor.matmul(
                                po3a[:], zst,
                                w2t[:, h, 3 * NMOV:3 * NMOV + HM],
                                start=h == 0, stop=h == HC - 1,
                            )
                            nc.tensor.matmul(
                                po3b[:], zst,
                                w2t[:, h, 3 * NMOV + HM:D],
                                start=h == 0, stop=h == HC - 1,
                            )
                        o3a = op.tile([P, HM], bf16, tag="o3a")
                        o3b = op.tile([P, HM], bf16, tag="o3b")
                        nc.scalar.copy(o3a[:], po3a[:])
                        nc.scalar.dma_start(
                            out[r0:r0 + P, 3 * NMOV:3 * NMOV + HM], o3a[:]
                        )
                        nc.vector.tensor_copy(o3b[:], po3b[:])
                        nc.sync.dma_start(
                            out[r0:r0 + P, 3 * NMOV + HM:D], o3b[:]
                        )

    nc.compile()
    return nc


def _get_program():
    if "nc" not in _CACHE:
        _CACHE["nc"] = _build_program()
    return _CACHE["nc"]


def _prep_w13(w1e, w3e):
    # [D, H] x2 -> [HC, P, 2, DC, P]; element [h,p,w,c,m] = w{1|3}[c*P+p, h*P+m]
    a = w1e.astype(ml_dtypes.bfloat16).reshape(DC, P, HC, P).transpose(2, 1, 0, 3)
    b = w3e.astype(ml_dtypes.bfloat16).reshape(DC, P, HC, P).transpose(2, 1, 0, 3)
    return np.ascontiguousarray(np.stack([a, b], axis=2))


def _numpy_fallback(x, w1, w2, w3, m_sizes):
    offs = np.concatenate([[0], np.cumsum(np.asarray(m_sizes, dtype=np.int64))])
    out = np.zeros((x.shape[0], w2.shape[2]), dtype=np.float32)
    for e in range(w1.shape[0]):
        xe = x[offs[e]:offs[e + 1]]
        u = xe @ w1[e]
        g = xe @ w3[e]
        z = (u / (1.0 + np.exp(-u))) * g
        out[offs[e]:offs[e + 1]] = z @ w2[e]
    return out


def kernel(x, w1, w2, w3, m_sizes, _trace=False, _trace_kwargs=None):
    global LAST_RESULTS
    x = np.ascontiguousarray(x, dtype=np.float32)
    w1 = np.ascontiguousarray(w1, dtype=np.float32)
    w2 = np.ascontiguousarray(w2, dtype=np.float32)
    w3 = np.ascontiguousarray(w3, dtype=np.float32)
    m = np.asarray(m_sizes, dtype=np.int64)

    expected = (
        x.shape == (T, D)
        and w1.shape == (E, D, H)
        and w2.shape == (E, H, D)
        and w3.shape == (E, D, H)
        and m.shape == (E,)
        and np.all(m == M)
    )
    if not expected:
        return _numpy_fallback(x, w1, w2, w3, m_sizes)

    from concourse.bass_utils import run_bass_kernel_spmd

    nc = _get_program()
    in_maps = []
    for e in range(E):
        in_maps.append({
            "xT": np.ascontiguousarray(
                x[e * M:(e + 1) * M].astype(ml_dtypes.bfloat16).T
            ),
            "w13r": _prep_w13(w1[e], w3[e]),
            "w2r": np.ascontiguousarray(
                w2[e].astype(ml_dtypes.bfloat16).reshape(HC, P, D)
            ),
        })

    res = run_bass_kernel_spmd(
        nc, in_maps, core_ids=list(range(E)),
        trace=_trace, **(_trace_kwargs or {}),
    )
    LAST_RESULTS = res
    return np.concatenate(
        [np.asarray(r["out"]) for r in res.results], axis=0
    ).astype(np.float32)
